# revision 41
# baseline (speedup 1.0000x reference)
"""Trainium2 Bass kernel for AttentionWithSharedWeights (LoRA attention, GQA, RoPE).

Sharding over 8 NeuronCores: batch (4) x head-group (2).  Each core computes
8 Q heads / 2 KV heads of one batch and a partial (head-sliced) output
projection; the host sums the two partials per batch.

Host-side preprocessing (inside kernel(), pure layout/reparameterization):
  - LoRA folded into dense weights (W_eff = W + B @ A)
  - x transposed per batch; weights transposed so every matmul operand is in
    its natural [contraction-dim-major] layout (no on-chip transposes)
  - x and the Q weights additionally quantized to fp8-e4m3 (wq scaled by 32,
    undone in the exp's activation scale) for the DoubleRow Q projection
  - RoPE rows pre-permuted (even dims then odd dims per head) so the rotation
    becomes a constant 128x128 matmul + two elementwise multiplies
  - cos/sin tables, causal masks, all-ones matrix precomputed

Device program (single SPMD program; Q projection in fp8-e4m3 DoubleRow =
2x PE rate with a 256-deep contraction per instruction; attention operands
in bf16; K/V projections and output projection in float32r = full PE rate):
  A) QKV projections from resident weights + streamed x.T chunks, fused RoPE;
     K (feature-major, bf16) and V (token-major, bf16) stay resident in SBUF,
     roped Q spills to DRAM in bf16.
  B) Causal attention in scores-transposed layout: S^T = K^T_tile.T @ Q^T,
     exp on the scalar engine (softmax + fp8 scale folded in, two k-tiles per
     activation), P@V accumulated in PSUM per 512-token q-chunk.  The softmax
     denominator sums each quad of exp tiles on the vector engine (bf16 2x
     mode), so one quarter-traffic allones matmul per quad accumulates in the
     parallel PSUM bank; one reciprocal + multiply normalizes at eviction.
     Diagonal tiles only compute their causal q-range; q tiles prefetch one
     (head, q-chunk) iteration ahead.
  RoPE is software-pipelined one projection group behind its matmuls; x
  streams through a 3-slot half-chunk ring with next-chunk prefetch (each
  transfer split across DMA queues), and DMA issue order front-loads what
  the first K projection needs.
  C) Output projection from resident wo, token-major y written straight out;
     ot tiles prefetch one q-chunk ahead.
"""

import numpy as np

B, S, DIM = 4, 2048, 2048
NH, NKV, HD = 16, 4, 128
LR = 16          # lora rank
SC = 512         # sequence chunk
NSC = S // SC    # 4
NKT = S // HD    # 16 k-tiles
HPC = NH // 2    # 8 q heads per core
KVPC = NKV // 2  # 2 kv heads per core
FQ = HPC * HD    # 1024 q features per core
FKV = KVPC * HD  # 256 kv features per core
SCALE = 1.0 / float(np.sqrt(HD))
WQS = 32.0     # host-side scale folded into the fp8 Q weights

_cache = {}


def _build_program():
    import concourse.mybir as mybir
    import concourse.tile as tile
    from concourse import bacc

    f32 = mybir.dt.float32
    f32r = mybir.dt.float32r
    bf16 = mybir.dt.bfloat16
    Exp = mybir.ActivationFunctionType.Exp

    f8 = mybir.dt.float8e4
    DR = mybir.MatmulPerfMode.DoubleRow

    nc = bacc.Bacc()

    # ---- DRAM parameters (per-core views, host-prepared layouts) ----
    xt_d = nc.declare_dram_parameter("xt", [DIM, S], f32r, isOutput=False)
    xt8_d = nc.declare_dram_parameter("xt8", [DIM, S], f8, isOutput=False)
    wq8_d = nc.declare_dram_parameter("wq8", [DIM, FQ], f8, isOutput=False)
    wk_d = nc.declare_dram_parameter("wk", [DIM, FKV], f32r, isOutput=False)
    wv_d = nc.declare_dram_parameter("wv", [DIM, FKV], f32r, isOutput=False)
    wo_d = nc.declare_dram_parameter("wo", [FQ, DIM], f32r, isOutput=False)
    cs_d = nc.declare_dram_parameter("cs", [HD, S], bf16, isOutput=False)
    sn_d = nc.declare_dram_parameter("sn", [HD, S], bf16, isOutput=False)
    rt_d = nc.declare_dram_parameter("rt", [HD, HD], f32r, isOutput=False)
    on_d = nc.declare_dram_parameter("on", [HD, HD], bf16, isOutput=False)
    mk_d = nc.declare_dram_parameter("mk", [HD, 4, SC], bf16, isOutput=False)
    y_d = nc.declare_dram_parameter("y", [S, DIM], bf16, isOutput=True)

    # internal spills, split per sequence chunk so cross-phase DRAM
    # dependencies are per-chunk rather than whole-tensor
    qt_ds = [nc.dram_tensor(f"qt_spill{i}", [FQ, SC], bf16) for i in range(NSC)]

    with tile.TileContext(nc) as tc:
        # K/V stay in SBUF across phases A and B: raw allocations so that
        # pool stacks of each phase bump above them without overlap.
        kt_t = nc.alloc_sbuf_tensor("kt_res", [HD, KVPC, S], bf16)
        v_t = nc.alloc_sbuf_tensor("v_res", [HD, NKT, FKV], bf16)
        kt_sb = kt_t[:]       # K feat-major, roped
        v_sb = v_t[:]         # V token-major
        if True:

            # ---------------- Phase A: projections + RoPE ----------------
            with tc.tile_pool(name="pa_w", bufs=1) as pw, \
                 tc.tile_pool(name="pa_x", bufs=1) as px, \
                 tc.tile_pool(name="pa_r", bufs=2) as pr, \
                 tc.tile_pool(name="pa_ps", bufs=4, space="PSUM") as pps, \
                 tc.tile_pool(name="pa_rot", bufs=2, space="PSUM") as prot, \
                 tc.tile_pool(name="pa_vps", bufs=2, space="PSUM") as pvps:

                wq_sb = pw.tile([HD, NKT, FQ], f8)
                wk_sb = pw.tile([HD, NKT, FKV], f32r)
                wv_sb = pw.tile([HD, NKT, FKV], f32r)
                rt_sb = pw.tile([HD, HD], f32r)

                # x chunks stream in half-chunk (8 k-tile) tiles through a
                # 3-slot ring so the next chunk's first half prefetches
                # under the current chunk's compute.  DMA issue order puts
                # what the first K-projection needs ahead of the bulky wq.
                pending = {}

                def load_half(s, half):
                    # split across DMA queues: one dma_start per 2 k-tiles
                    xh = px.tile([HD, 8, SC], f32r, tag="xc", bufs=3,
                                 name=f"xc{s}_{half}")
                    k0 = half * 8
                    for kg in range(0, 8, 2):
                        nc.sync.dma_start(
                            xh[:, kg:kg + 2, :],
                            xt_d[(k0 + kg) * HD:(k0 + kg + 2) * HD,
                                 s * SC:(s + 1) * SC].rearrange(
                                "(k p) s -> p k s", p=HD))
                    return xh

                xh00 = px.tile([HD, 8, SC], f32r, tag="xc", bufs=3,
                               name="xc0_0")
                for kg in range(0, 8, 2):
                    nc.sync.dma_start(
                        xh00[:, kg:kg + 2, :],
                        xt_d[kg * HD:(kg + 2) * HD, 0:SC].rearrange(
                            "(k p) s -> p k s", p=HD))
                    nc.sync.dma_start(
                        wk_sb[:, 4 * (kg // 2):4 * (kg // 2) + 4, :],
                        wk_d[4 * (kg // 2) * HD:(4 * (kg // 2) + 4) * HD,
                             :].rearrange("(k p) f -> p k f", p=HD))
                pending[(0, 0)] = xh00
                nc.sync.dma_start(rt_sb[:], rt_d[:])
                pending[(0, 1)] = load_half(0, 1)
                nc.sync.dma_start(wv_sb[:], wv_d[:].rearrange("(k p) f -> p k f", p=HD))
                for ft in range(HPC):
                    nc.sync.dma_start(
                        wq_sb[:, :, ft * HD:(ft + 1) * HD],
                        wq8_d[:, ft * HD:(ft + 1) * HD].rearrange(
                            "(k p) f -> p k f", p=HD))

                # fp8 copy of x for the DoubleRow Q projection, streamed
                # whole-chunk (it is only needed at the end of each chunk)
                def load_x8(s):
                    x8 = px.tile([HD, NKT, SC], f8, tag="x8", bufs=2,
                                 name=f"x8_{s}")
                    for kg in range(0, NKT, 8):
                        nc.sync.dma_start(
                            x8[:, kg:kg + 8, :],
                            xt8_d[kg * HD:(kg + 8) * HD,
                                  s * SC:(s + 1) * SC].rearrange(
                                "(k p) s -> p k s", p=HD))
                    return x8

                pending[(0, "x8")] = load_x8(0)

                for sc in range(NSC):
                    ssl = slice(sc * SC, (sc + 1) * SC)
                    xlo = pending.pop((sc, 0))
                    xhi = pending.pop((sc, 1))
                    x8c = pending.pop((sc, "x8"))
                    cs_sb = px.tile([HD, SC], bf16, tag="cs", bufs=2)
                    sn_sb = px.tile([HD, SC], bf16, tag="sn", bufs=2)
                    nc.sync.dma_start(cs_sb[:], cs_d[:, ssl])
                    nc.sync.dma_start(sn_sb[:], sn_d[:, ssl])

                    def xck(kt):
                        return (xlo if kt < 8 else xhi)[:, kt % 8, :]

                    # RoPE is software-pipelined one projection group
                    # behind: the PSUM->SBUF copy drains while the next
                    # group's matmuls run, so the rotation matmul and its
                    # DVE tail never stall the PE.
                    rope_q = []

                    def rope_start(raw_ps, out_ap, spill=None):
                        raw = pr.tile([HD, SC], f32r, tag="rope_raw")
                        nc.any.tensor_copy(out=raw[:], in_=raw_ps[:])
                        rope_q.append((raw, out_ap, spill))

                    def rope_flush():
                        if not rope_q:
                            return
                        raw, out_ap, spill = rope_q.pop(0)
                        rot_ps = prot.tile([HD, SC], f32)
                        nc.tensor.matmul(rot_ps[:], rt_sb[:], raw[:],
                                         start=True, stop=True)
                        tmp = pr.tile([HD, SC], f32, tag="rope_tmp")
                        nc.vector.tensor_mul(tmp[:], raw[:].bitcast(f32),
                                             cs_sb[:])
                        e1 = pr.tile([HD, SC], f32, tag="rope_e1")
                        nc.vector.tensor_mul(e1[:], rot_ps[:], sn_sb[:])
                        nc.vector.tensor_add(out_ap, tmp[:], e1[:])
                        if spill is not None:
                            nc.sync.dma_start(spill, out_ap)

                    # K projection + rope -> resident SBUF
                    for ft in range(KVPC):
                        fsl = slice(ft * HD, (ft + 1) * HD)
                        k_ps = pps.tile([HD, SC], f32, tag="qk_ps")
                        for kt in range(NKT):
                            nc.tensor.matmul(k_ps[:], wk_sb[:, kt, fsl], xck(kt),
                                             start=(kt == 0), stop=(kt == NKT - 1))
                        if ft > 0:
                            rope_flush()
                        rope_start(k_ps, kt_sb[:, ft, ssl])

                    # V projection, token-major -> resident SBUF
                    for st in range(SC // HD):
                        tsl = slice(st * HD, (st + 1) * HD)
                        v_ps = pvps.tile([HD, FKV], f32, tag="v_ps")
                        for kt in range(NKT):
                            nc.tensor.matmul(v_ps[:], xck(kt)[:, tsl], wv_sb[:, kt, :],
                                             start=(kt == 0), stop=(kt == NKT - 1))
                        rope_flush()
                        nc.any.tensor_copy(
                            out=v_sb[:, sc * (SC // HD) + st, :], in_=v_ps[:])

                    # prefetch next chunk's first half into the free slot
                    if sc + 1 < NSC:
                        pending[(sc + 1, 0)] = load_half(sc + 1, 0)

                    # Q projection (fp8 DoubleRow: 256-deep contraction per
                    # instruction at ~2x PE rate) + rope -> spill to DRAM
                    for ft in range(HPC):
                        fsl = slice(ft * HD, (ft + 1) * HD)
                        q_ps = pps.tile([HD, SC], f32, tag="qk_ps")
                        for kp in range(NKT // 2):
                            nc.tensor.matmul(
                                q_ps[:], wq_sb[:, 2 * kp:2 * kp + 2, fsl],
                                x8c[:, 2 * kp:2 * kp + 2, :],
                                start=(kp == 0), stop=(kp == NKT // 2 - 1),
                                perf_mode=DR)
                        rope_flush()
                        qfin = pr.tile([HD, SC], bf16, tag="qfin", bufs=4)
                        rope_start(q_ps, qfin[:], spill=qt_ds[sc][fsl, :])
                    rope_flush()

                    if sc + 1 < NSC:
                        pending[(sc + 1, 1)] = load_half(sc + 1, 1)
                        pending[(sc + 1, "x8")] = load_x8(sc + 1)

        # Phase C weights: pool opened before phase B so the wo load (8.4 MB)
        # overlaps attention compute.
        with tc.tile_pool(name="pc_w", bufs=1) as pcw:
            wo_sb = pcw.tile([HD, HPC, DIM], f32r)

            # ------- Phase B+C fused: causal attention + out-projection -----
            # q-chunk outer, head-pair inner; each chunk's attention output
            # stays resident in SBUF and its output projection is emitted
            # right behind it, so the PE drains the scalar engine's exp
            # backlog with projection matmuls instead of idling, and the
            # ot spill/reload round-trip disappears.
            with tc.tile_pool(name="pb_c", bufs=1) as pbc, \
                 tc.tile_pool(name="pb_q", bufs=4) as pbq, \
                 tc.tile_pool(name="pb_e", bufs=8) as pbe, \
                 tc.tile_pool(name="pb_o", bufs=3) as pbo, \
                 tc.tile_pool(name="pb_sps", bufs=2, space="PSUM") as sps, \
                 tc.tile_pool(name="pb_ops", bufs=2, space="PSUM") as ops, \
                 tc.tile_pool(name="pb_bps", bufs=2, space="PSUM") as bps:

                # qt tiles are prefetched one (qc, h0) iteration ahead so the
                # first scores matmul of each block never waits on the DMA
                qt_pending = {}

                def load_qt(h, qc):
                    qt = pbq.tile([HD, SC], bf16, tag="qt",
                                  name=f"qt{h}_{qc}")
                    nc.sync.dma_start(
                        qt[:], qt_ds[qc][h * HD:(h + 1) * HD, :])
                    qt_pending[(h, qc)] = qt

                for h in (0, 1):
                    load_qt(h, 0)
                on_sb = pbc.tile([HD, HD], bf16)
                mk_sb = pbc.tile([HD, HD], bf16)
                nc.sync.dma_start(on_sb[:], on_d[:])
                nc.sync.dma_start(mk_sb[:], mk_d[:, 0, 0:HD])
                for qc in range(NSC):
                    otr = pbo.tile([HD, HPC, SC], f32r, tag="ot_res", bufs=2,
                                   name=f"otr{qc}")
                    for h0 in range(0, HPC, 2):
                        hs = (h0, h0 + 1)
                        kv = h0 // (HPC // KVPC)
                        if qc == 0:
                            # spread the wo load across the first chunk's
                            # blocks; all of it lands before C(0) needs it
                            for h in hs:
                                nc.sync.dma_start(
                                    wo_sb[:, h, :],
                                    wo_d[h * HD:(h + 1) * HD, :])
                        nkt = 4 * qc + 4
                        npairs = nkt // 2
                        qts, ots, bcs, es, epq = {}, {}, {}, {}, {}
                        for h in hs:
                            qts[h] = qt_pending.pop((h, qc))
                            ots[h] = ops.tile([HD, SC], f32, tag="ot_ps",
                                              name=f"ot{h}")
                            bcs[h] = bps.tile([HD, SC], f32, tag="bc_ps",
                                              name=f"bc{h}")
                        # prefetch the next iteration's q tiles
                        nqc, nh0 = (qc, h0 + 2) if h0 + 2 < HPC else (qc + 1, 0)
                        if nqc < NSC:
                            for h in (nh0, nh0 + 1):
                                load_qt(h, nqc)

                        def b_scores(h, kp):
                            """scores + exp (+mask) for k-tile pair kp."""
                            kt0, kt1 = 2 * kp, 2 * kp + 1
                            qt = qts[h]
                            s_ps = sps.tile([HD, 2, SC], f32, tag="s_ps")
                            e = pbe.tile([HD, 2, SC], bf16, tag="e")
                            es[(h, kp)] = e
                            if kt1 < 4 * qc:
                                # off-diagonal pair: full width, one exp
                                for i, kt in enumerate((kt0, kt1)):
                                    nc.tensor.matmul(
                                        s_ps[:, i, :],
                                        kt_sb[:, kv, kt * HD:(kt + 1) * HD],
                                        qt[:], start=True, stop=True)
                                nc.scalar.activation(e[:], s_ps[:], Exp,
                                                     scale=SCALE / WQS)
                            else:
                                for i, kt in enumerate((kt0, kt1)):
                                    q0 = (kt - 4 * qc) * HD
                                    nc.tensor.matmul(
                                        s_ps[:, i, q0:],
                                        kt_sb[:, kv, kt * HD:(kt + 1) * HD],
                                        qt[:, q0:], start=True, stop=True)
                                    nc.scalar.activation(
                                        e[:, i, q0:], s_ps[:, i, q0:], Exp,
                                        scale=SCALE / WQS)
                                    # intra-tile triangle mask (in place)
                                    nc.vector.tensor_mul(
                                        e[:, i, q0:q0 + HD],
                                        e[:, i, q0:q0 + HD],
                                        mk_sb[:])

                        def b_accum(h, kp):
                            """P@V + denominator accumulation for pair kp.

                            Denominator: DVE sums four e-tiles (one quad = two
                            pairs) into one bf16 ep, so the ones-matmul runs
                            once per quad at full width; quads never straddle
                            the diagonal (it starts at kt = 4*qc, a multiple
                            of 4), so only the diagonal quad needs ranged adds.
                            """
                            kt0, kt1 = 2 * kp, 2 * kp + 1
                            e = es.pop((h, kp))
                            for i, kt in enumerate((kt0, kt1)):
                                q0 = max(0, (kt - 4 * qc) * HD)
                                nc.tensor.matmul(
                                    ots[h][:, q0:],
                                    v_sb[:, kt, kv * HD:(kv + 1) * HD],
                                    e[:, i, q0:], start=(kt == 0),
                                    stop=(kt == nkt - 1),
                                    skip_group_check=True)
                            g0 = max(0, (kt0 - 4 * qc) * HD)
                            g1 = max(0, (kt1 - 4 * qc) * HD)
                            if kp % 2 == 0:
                                ep = pbe.tile([HD, SC], bf16, tag="ep")
                                epq[h] = ep
                                if g1 == 0:
                                    nc.vector.tensor_add(
                                        ep[:], e[:, 0, :], e[:, 1, :])
                                else:
                                    nc.vector.tensor_copy(
                                        out=ep[:, g0:], in_=e[:, 0, g0:])
                                    nc.vector.tensor_add(
                                        ep[:, g1:], ep[:, g1:], e[:, 1, g1:])
                            else:
                                ep = epq.pop(h)
                                nc.vector.tensor_add(
                                    ep[:, g0:], ep[:, g0:], e[:, 0, g0:])
                                nc.vector.tensor_add(
                                    ep[:, g1:], ep[:, g1:], e[:, 1, g1:])
                                nc.tensor.matmul(
                                    bcs[h][:], on_sb[:], ep[:],
                                    start=kp == 1, stop=kp == npairs - 1,
                                    skip_group_check=True)

                        # two heads interleaved, scores one pair ahead: while
                        # the scalar engine exps one head's pair, the PE runs
                        # the other head's matmuls
                        b_scores(h0, 0)
                        b_scores(h0 + 1, 0)
                        for kp in range(npairs):
                            for h in hs:
                                if kp + 1 < npairs:
                                    b_scores(h, kp + 1)
                                b_accum(h, kp)

                        for h in hs:
                            inv = pbo.tile([HD, SC], f32, tag="inv")
                            nc.vector.reciprocal(inv[:], bcs[h][:])
                            nc.vector.tensor_mul(otr[:, h, :], ots[h][:],
                                                 inv[:])

                    # ---- output projection for this q-chunk (fused) ----
                    # y_ps shares the s_ps psum tag; y accumulates a full
                    # [128, DIM] row block in SBUF so the store is one DMA
                    # per q-tile
                    for qs in range(SC // HD):
                        qt0 = qc * SC + qs * HD
                        y_sb = pbo.tile([HD, DIM], bf16, tag="y_sb", bufs=2)
                        for dc in range(DIM // SC):
                            dsl = slice(dc * SC, (dc + 1) * SC)
                            y_ps = sps.tile([HD, 2, SC], f32, tag="s_ps",
                                            name=f"y{qs}_{dc}")
                            for ft in range(HPC):
                                nc.tensor.matmul(
                                    y_ps[:, 0, :],
                                    otr[:, ft, qs * HD:(qs + 1) * HD],
                                    wo_sb[:, ft, dsl],
                                    start=(ft == 0), stop=(ft == HPC - 1))
                            nc.any.tensor_copy(out=y_sb[:, dsl],
                                               in_=y_ps[:, 0, :])
                        nc.sync.dma_start(y_d[qt0:qt0 + HD, :], y_sb[:])

    nc.finalize()
    return nc


def _rope_perm(nheads):
    """Row permutation putting even dims first within each head."""
    idx = []
    for h in range(nheads):
        base = h * HD
        idx.extend(base + 2 * j for j in range(HD // 2))
        idx.extend(base + 2 * j + 1 for j in range(HD // 2))
    return np.array(idx)


def _prepare_in_maps(inputs):
    import ml_dtypes
    ml_bf16 = ml_dtypes.bfloat16
    ml_f8 = ml_dtypes.float8_e4m3
    x = np.ascontiguousarray(np.asarray(inputs["x"], dtype=np.float32))
    fc = np.asarray(inputs["freqs_cos"], dtype=np.float32)
    fs = np.asarray(inputs["freqs_sin"], dtype=np.float32)
    wq = np.asarray(inputs["wq"], dtype=np.float32)
    wk = np.asarray(inputs["wk"], dtype=np.float32)
    wv = np.asarray(inputs["wv"], dtype=np.float32)
    wo = np.asarray(inputs["wo"], dtype=np.float32)
    aq = np.asarray(inputs["aq"], dtype=np.float32)
    bq = np.asarray(inputs["bq"], dtype=np.float32)
    ak = np.asarray(inputs["ak"], dtype=np.float32)
    bk = np.asarray(inputs["bk"], dtype=np.float32)
    av = np.asarray(inputs["av"], dtype=np.float32)
    bv = np.asarray(inputs["bv"], dtype=np.float32)
    ao = np.asarray(inputs["ao"], dtype=np.float32)
    bo = np.asarray(inputs["bo"], dtype=np.float32)

    permQ = _rope_perm(HPC)
    permK = _rope_perm(KVPC)
    # fold LoRA into dense weights: W_eff = W + B @ A
    wq = wq + bq.astype(np.float64) @ aq.astype(np.float64)
    wk = wk + bk.astype(np.float64) @ ak.astype(np.float64)
    wv = wv + bv.astype(np.float64) @ av.astype(np.float64)
    wo = wo + bo.astype(np.float64) @ ao.astype(np.float64)
    wq = wq.astype(np.float32)
    wk = wk.astype(np.float32)
    wv = wv.astype(np.float32)
    wo = wo.astype(np.float32)
    fcT = np.ascontiguousarray(fc.T)                       # [64, S]
    fsT = np.ascontiguousarray(fs.T)
    cs = np.concatenate([fcT, fcT], axis=0)                # [128, S]
    sn = np.concatenate([fsT, fsT], axis=0)
    rt = np.zeros((HD, HD), np.float32)
    for j in range(HD // 2):
        rt[j, 64 + j] = 1.0      # (R^T)[j, 64+j] = R[64+j, j] = +1
        rt[64 + j, j] = -1.0     # (R^T)[64+j, j] = R[j, 64+j] = -1
    ones = np.ones((HD, HD), np.float32)
    kk = np.arange(HD)[:, None]
    qq = np.arange(SC)[None, :]
    mk = np.stack([(qq >= (128 * r + kk)).astype(np.float32) for r in range(4)],
                  axis=1)                                  # [128, 4, SC]

    xt_cache = {}
    xt8_cache = {}
    in_maps = []
    for c in range(8):
        b, g = c // 2, c % 2
        if b not in xt_cache:
            xt_cache[b] = np.ascontiguousarray(x[b].T)
            xt8_cache[b] = np.clip(xt_cache[b], -240.0, 240.0).astype(ml_f8)
        fq = slice(g * FQ, (g + 1) * FQ)
        fkv = slice(g * FKV, (g + 1) * FKV)
        wq_g = wq[fq][permQ]
        wk_g = wk[fkv][permK]
        in_maps.append({
            "xt": xt_cache[b],
            "xt8": xt8_cache[b],
            "wq8": np.clip(np.ascontiguousarray(wq_g.T) * WQS,
                           -240.0, 240.0).astype(ml_f8),
            "wk": np.ascontiguousarray(wk_g.T),
            "wv": np.ascontiguousarray(wv[fkv].T),
            "wo": np.ascontiguousarray(wo[:, fq].T),
            "cs": cs.astype(ml_bf16), "sn": sn.astype(ml_bf16),
            "rt": rt, "on": ones.astype(ml_bf16), "mk": mk.astype(ml_bf16),
        })
    return in_maps


def _get_program():
    if "nc" not in _cache:
        _cache["nc"] = _build_program()
    return _cache["nc"]


def run(inputs, trace=False):
    from concourse import bass_utils
    nc = _get_program()
    in_maps = _prepare_in_maps(inputs)
    res = bass_utils.run_bass_kernel_spmd(
        nc, in_maps, list(range(8)), trace=trace)
    ys = [res.results[c]["y"] for c in range(8)]
    out = np.empty((B, S, DIM), np.float32)
    for b in range(B):
        out[b] = ys[2 * b] + ys[2 * b + 1]
    return out, res


def kernel(**inputs):
    out, _ = run(inputs, trace=False)
    return out


def bench_chain(inputs, nlo=4, nhi=36, rounds=7, n_cores=8):
    """Marginal per-execution device time via chained NEFF executions.

    Builds one jitted program that runs the kernel N times back-to-back on
    device (each execution's output buffers feed the next, forcing serial
    order), so the host/axon dispatch overhead is paid once per program.
    Marginal time (wall(nhi) - wall(nlo)) / (nhi - nlo) is the true
    per-execution time.  Returns (median_marginal_seconds, output).
    """
    import time

    import jax
    import concourse.mybir as mybir
    from concourse import bass2jax
    from concourse.bass2jax import _bass_exec_p, partition_id_tensor
    from jax.sharding import Mesh, NamedSharding, PartitionSpec
    from jax.experimental.shard_map import shard_map

    bass2jax.install_neuronx_cc_hook()
    nc = _get_program()
    in_maps = _prepare_in_maps(inputs)

    partition_name = nc.partition_id_tensor.name if nc.partition_id_tensor else None
    in_names, out_names, out_avals = [], [], []
    for alloc in nc.m.functions[0].allocations:
        if not isinstance(alloc, mybir.MemoryLocationSet):
            continue
        name = alloc.memorylocations[0].name
        if alloc.kind == "ExternalInput":
            if name != partition_name:
                in_names.append(name)
        elif alloc.kind == "ExternalOutput":
            out_names.append(name)
            out_avals.append(jax.core.ShapedArray(
                tuple(alloc.tensor_shape), mybir.dt.np(alloc.dtype)))
    n_params = len(in_names)
    all_names = list(in_names) + out_names
    if partition_name is not None:
        all_names.append(partition_name)

    devices = jax.devices()[:n_cores]
    mesh = Mesh(np.asarray(devices), ("core",))
    spec = NamedSharding(mesh, PartitionSpec("core"))

    def make(n):
        def _body(*args):
            ins = list(args[:n_params])
            outs = list(args[n_params:])
            for _ in range(n):
                operands = ins + outs
                if partition_name is not None:
                    operands.append(partition_id_tensor())
                outs = list(_bass_exec_p.bind(
                    *operands,
                    out_avals=tuple(out_avals),
                    in_names=tuple(all_names),
                    out_names=tuple(out_names),
                    lowering_input_output_aliases=(),
                    sim_require_finite=True,
                    sim_require_nnan=True,
                    nc=nc,
                ))
            return tuple(outs)

        return jax.jit(shard_map(
            _body, mesh=mesh,
            in_specs=(PartitionSpec("core"),) * (n_params + len(out_names)),
            out_specs=(PartitionSpec("core"),) * len(out_names),
            check_rep=False), keep_unused=True)

    concat_in = [
        jax.device_put(
            np.concatenate([np.asarray(in_maps[c][nm]) for c in range(n_cores)],
                           axis=0), spec)
        for nm in in_names]
    concat_zeros = [
        jax.device_put(
            np.zeros((n_cores * a.shape[0], *a.shape[1:]), a.dtype), spec)
        for a in out_avals]

    flo, fhi = make(nlo), make(nhi)
    out = flo(*concat_in, *concat_zeros)
    jax.block_until_ready(out)
    jax.block_until_ready(fhi(*concat_in, *concat_zeros))

    margs = []
    for _ in range(rounds):
        t0 = time.perf_counter()
        jax.block_until_ready(fhi(*concat_in, *concat_zeros))
        t1 = time.perf_counter()
        jax.block_until_ready(flo(*concat_in, *concat_zeros))
        t2 = time.perf_counter()
        margs.append(((t1 - t0) - (t2 - t1)) / (nhi - nlo))
    med = float(np.median(margs))

    ys = np.asarray(out[out_names.index("y")]).reshape(n_cores, S, DIM)
    full = np.empty((B, S, DIM), np.float32)
    for b in range(B):
        full[b] = ys[2 * b] + ys[2 * b + 1]
    return med, full, [round(m * 1e9) for m in margs]


def bench(inputs, iters=20, n_cores=8):
    """Time repeated NEFF executions with device-resident inputs.

    Mirrors bass2jax.run_bass_via_pjrt's multi-core path without donation so
    buffers can be reused across calls.  Returns (avg_exec_seconds, output).
    """
    import time

    import jax
    import concourse.mybir as mybir
    from concourse import bass2jax
    from concourse.bass2jax import _bass_exec_p, partition_id_tensor
    from jax.sharding import Mesh, NamedSharding, PartitionSpec

    bass2jax.install_neuronx_cc_hook()
    nc = _get_program()
    in_maps = _prepare_in_maps(inputs)

    partition_name = nc.partition_id_tensor.name if nc.partition_id_tensor else None
    in_names, out_names, out_avals = [], [], []
    for alloc in nc.m.functions[0].allocations:
        if not isinstance(alloc, mybir.MemoryLocationSet):
            continue
        name = alloc.memorylocations[0].name
        if alloc.kind == "ExternalInput":
            if name != partition_name:
                in_names.append(name)
        elif alloc.kind == "ExternalOutput":
            out_names.append(name)
            out_avals.append(jax.core.ShapedArray(
                tuple(alloc.tensor_shape), mybir.dt.np(alloc.dtype)))
    n_params = len(in_names)
    all_names = list(in_names) + out_names
    if partition_name is not None:
        all_names.append(partition_name)

    def _body(*args):
        operands = list(args)
        if partition_name is not None:
            operands.append(partition_id_tensor())
        outs = _bass_exec_p.bind(
            *operands,
            out_avals=tuple(out_avals),
            in_names=tuple(all_names),
            out_names=tuple(out_names),
            lowering_input_output_aliases=(),
            sim_require_finite=True,
            sim_require_nnan=True,
            nc=nc,
        )
        return tuple(outs)

    devices = jax.devices()[:n_cores]
    mesh = Mesh(np.asarray(devices), ("core",))
    spec = NamedSharding(mesh, PartitionSpec("core"))
    from jax.experimental.shard_map import shard_map
    sharded = jax.jit(shard_map(
        _body, mesh=mesh,
        in_specs=(PartitionSpec("core"),) * (n_params + len(out_names)),
        out_specs=(PartitionSpec("core"),) * len(out_names),
        check_rep=False), keep_unused=True)

    concat_in = [
        jax.device_put(
            np.concatenate([np.asarray(in_maps[c][nm]) for c in range(n_cores)],
                           axis=0), spec)
        for nm in in_names]
    concat_zeros = [
        jax.device_put(
            np.zeros((n_cores * a.shape[0], *a.shape[1:]), a.dtype), spec)
        for a in out_avals]
    out = sharded(*concat_in, *concat_zeros)
    jax.block_until_ready(out)
    t0 = time.perf_counter()
    for _ in range(iters):
        out = sharded(*concat_in, *concat_zeros)
    jax.block_until_ready(out)
    t1 = time.perf_counter()

    if n_cores != 8:
        return (t1 - t0) / iters, None
    ys = np.asarray(out[out_names.index("y")]).reshape(n_cores, S, DIM)
    full = np.empty((B, S, DIM), np.float32)
    for b in range(B):
        full[b] = ys[2 * b] + ys[2 * b + 1]
    return (t1 - t0) / iters, full





# revision 44
# speedup vs baseline: 2.2758x; 2.2758x over previous
"""Trainium2 Bass kernel for AttentionWithSharedWeights (LoRA attention, GQA, RoPE).

Sharding over 8 NeuronCores: batch (4) x head-group (2).  Each core computes
8 Q heads / 2 KV heads of one batch and a partial (head-sliced) output
projection; the host sums the two partials per batch.

Host-side preprocessing (inside kernel(), pure layout/reparameterization):
  - LoRA folded into dense weights (W_eff = W + B @ A)
  - x transposed per batch; weights transposed so every matmul operand is in
    its natural [contraction-dim-major] layout (no on-chip transposes)
  - x and the Q weights additionally quantized to fp8-e4m3 (wq scaled by 32,
    undone in the exp's activation scale) for the DoubleRow Q projection
  - RoPE rows pre-permuted (even dims then odd dims per head) so the rotation
    becomes a constant 128x128 matmul + two elementwise multiplies
  - cos/sin tables, causal masks, all-ones matrix precomputed

Device program (single SPMD program; Q projection in fp8-e4m3 DoubleRow =
2x PE rate with a 256-deep contraction per instruction; attention operands
in bf16; K/V projections and output projection in float32r = full PE rate):
  A) QKV projections from resident weights + streamed x.T chunks, fused RoPE;
     K (feature-major, bf16) and V (token-major, bf16) stay resident in SBUF,
     roped Q spills to DRAM in bf16.
  B) Causal attention in scores-transposed layout: S^T = K^T_tile.T @ Q^T,
     exp on the scalar engine (softmax + fp8 scale folded in, two k-tiles per
     activation), P@V accumulated in PSUM per 512-token q-chunk.  The softmax
     denominator sums each quad of exp tiles on the vector engine (bf16 2x
     mode), so one quarter-traffic allones matmul per quad accumulates in the
     parallel PSUM bank; one reciprocal + multiply normalizes at eviction.
     Diagonal tiles only compute their causal q-range; q tiles prefetch one
     (q-chunk, head) iteration ahead.
  RoPE is software-pipelined one projection group behind its matmuls; x
  streams through a 3-slot half-chunk ring with next-chunk prefetch (each
  transfer split across DMA queues), and DMA issue order front-loads what
  the first K projection needs.
  C) Fused into B per q-chunk: the chunk's attention output stays resident
     in SBUF (no DRAM round-trip) and its output projection from resident wo
     is emitted right behind the chunk's attention, sharing the scores PSUM
     ring; token-major y row blocks written straight out.
"""

import numpy as np

B, S, DIM = 4, 2048, 2048
NH, NKV, HD = 16, 4, 128
LR = 16          # lora rank
SC = 512         # sequence chunk
NSC = S // SC    # 4
NKT = S // HD    # 16 k-tiles
HPC = NH // 2    # 8 q heads per core
KVPC = NKV // 2  # 2 kv heads per core
FQ = HPC * HD    # 1024 q features per core
FKV = KVPC * HD  # 256 kv features per core
SCALE = 1.0 / float(np.sqrt(HD))
WQS = 32.0     # host-side scale folded into the fp8 Q weights

_cache = {}


def _build_program():
    import concourse.mybir as mybir
    import concourse.tile as tile
    from concourse import bacc

    f32 = mybir.dt.float32
    f32r = mybir.dt.float32r
    bf16 = mybir.dt.bfloat16
    Exp = mybir.ActivationFunctionType.Exp

    f8 = mybir.dt.float8e4
    DR = mybir.MatmulPerfMode.DoubleRow

    nc = bacc.Bacc()

    # ---- DRAM parameters (per-core views, host-prepared layouts) ----
    xt_d = nc.declare_dram_parameter("xt", [DIM, S], f32r, isOutput=False)
    xt8_d = nc.declare_dram_parameter("xt8", [DIM, S], f8, isOutput=False)
    wq8_d = nc.declare_dram_parameter("wq8", [DIM, FQ], f8, isOutput=False)
    wk_d = nc.declare_dram_parameter("wk", [DIM, FKV], f32r, isOutput=False)
    wv_d = nc.declare_dram_parameter("wv", [DIM, FKV], f32r, isOutput=False)
    wo_d = nc.declare_dram_parameter("wo", [FQ, DIM], f32r, isOutput=False)
    cs_d = nc.declare_dram_parameter("cs", [HD, S], bf16, isOutput=False)
    sn_d = nc.declare_dram_parameter("sn", [HD, S], bf16, isOutput=False)
    rt_d = nc.declare_dram_parameter("rt", [HD, HD], f32r, isOutput=False)
    on_d = nc.declare_dram_parameter("on", [HD, HD], bf16, isOutput=False)
    mk_d = nc.declare_dram_parameter("mk", [HD, 4, SC], bf16, isOutput=False)
    y_d = nc.declare_dram_parameter("y", [S, DIM], bf16, isOutput=True)

    # internal spills, split per sequence chunk so cross-phase DRAM
    # dependencies are per-chunk rather than whole-tensor
    qt_ds = [nc.dram_tensor(f"qt_spill{i}", [FQ, SC], bf16) for i in range(NSC)]

    with tile.TileContext(nc) as tc:
        # K/V stay in SBUF across phases A and B: raw allocations so that
        # pool stacks of each phase bump above them without overlap.
        kt_t = nc.alloc_sbuf_tensor("kt_res", [HD, KVPC, S], bf16)
        v_t = nc.alloc_sbuf_tensor("v_res", [HD, NKT, FKV], bf16)
        kt_sb = kt_t[:]       # K feat-major, roped
        v_sb = v_t[:]         # V token-major
        if True:

            # ---------------- Phase A: projections + RoPE ----------------
            with tc.tile_pool(name="pa_w", bufs=1) as pw, \
                 tc.tile_pool(name="pa_x", bufs=1) as px, \
                 tc.tile_pool(name="pa_r", bufs=2) as pr, \
                 tc.tile_pool(name="pa_ps", bufs=4, space="PSUM") as pps, \
                 tc.tile_pool(name="pa_rot", bufs=2, space="PSUM") as prot, \
                 tc.tile_pool(name="pa_vps", bufs=2, space="PSUM") as pvps:

                wq_sb = pw.tile([HD, NKT, FQ], f8)
                wk_sb = pw.tile([HD, NKT, FKV], f32r)
                wv_sb = pw.tile([HD, NKT, FKV], f32r)
                rt_sb = pw.tile([HD, HD], f32r)

                # x chunks stream in half-chunk (8 k-tile) tiles through a
                # 3-slot ring so the next chunk's first half prefetches
                # under the current chunk's compute.  DMA issue order puts
                # what the first K-projection needs ahead of the bulky wq.
                pending = {}

                def load_half(s, half):
                    # split across DMA queues: one dma_start per 2 k-tiles
                    xh = px.tile([HD, 8, SC], f32r, tag="xc", bufs=3,
                                 name=f"xc{s}_{half}")
                    k0 = half * 8
                    for kg in range(0, 8, 2):
                        nc.sync.dma_start(
                            xh[:, kg:kg + 2, :],
                            xt_d[(k0 + kg) * HD:(k0 + kg + 2) * HD,
                                 s * SC:(s + 1) * SC].rearrange(
                                "(k p) s -> p k s", p=HD))
                    return xh

                xh00 = px.tile([HD, 8, SC], f32r, tag="xc", bufs=3,
                               name="xc0_0")
                for kg in range(0, 8, 2):
                    nc.sync.dma_start(
                        xh00[:, kg:kg + 2, :],
                        xt_d[kg * HD:(kg + 2) * HD, 0:SC].rearrange(
                            "(k p) s -> p k s", p=HD))
                    nc.sync.dma_start(
                        wk_sb[:, 4 * (kg // 2):4 * (kg // 2) + 4, :],
                        wk_d[4 * (kg // 2) * HD:(4 * (kg // 2) + 4) * HD,
                             :].rearrange("(k p) f -> p k f", p=HD))
                pending[(0, 0)] = xh00
                nc.sync.dma_start(rt_sb[:], rt_d[:])
                pending[(0, 1)] = load_half(0, 1)
                nc.sync.dma_start(wv_sb[:], wv_d[:].rearrange("(k p) f -> p k f", p=HD))
                for ft in range(HPC):
                    nc.sync.dma_start(
                        wq_sb[:, :, ft * HD:(ft + 1) * HD],
                        wq8_d[:, ft * HD:(ft + 1) * HD].rearrange(
                            "(k p) f -> p k f", p=HD))

                # fp8 copy of x for the DoubleRow Q projection, streamed
                # whole-chunk (it is only needed at the end of each chunk)
                def load_x8(s):
                    x8 = px.tile([HD, NKT, SC], f8, tag="x8", bufs=2,
                                 name=f"x8_{s}")
                    for kg in range(0, NKT, 8):
                        nc.sync.dma_start(
                            x8[:, kg:kg + 8, :],
                            xt8_d[kg * HD:(kg + 8) * HD,
                                  s * SC:(s + 1) * SC].rearrange(
                                "(k p) s -> p k s", p=HD))
                    return x8

                pending[(0, "x8")] = load_x8(0)

                for sc in range(NSC):
                    ssl = slice(sc * SC, (sc + 1) * SC)
                    xlo = pending.pop((sc, 0))
                    xhi = pending.pop((sc, 1))
                    x8c = pending.pop((sc, "x8"))
                    cs_sb = px.tile([HD, SC], bf16, tag="cs", bufs=2)
                    sn_sb = px.tile([HD, SC], bf16, tag="sn", bufs=2)
                    nc.sync.dma_start(cs_sb[:], cs_d[:, ssl])
                    nc.sync.dma_start(sn_sb[:], sn_d[:, ssl])

                    def xck(kt):
                        return (xlo if kt < 8 else xhi)[:, kt % 8, :]

                    # RoPE is software-pipelined one projection group
                    # behind: the PSUM->SBUF copy drains while the next
                    # group's matmuls run, so the rotation matmul and its
                    # DVE tail never stall the PE.
                    rope_q = []

                    def rope_start(raw_ps, out_ap, spill=None):
                        raw = pr.tile([HD, SC], f32r, tag="rope_raw")
                        nc.any.tensor_copy(out=raw[:], in_=raw_ps[:])
                        rope_q.append((raw, out_ap, spill))

                    def rope_flush():
                        if not rope_q:
                            return
                        raw, out_ap, spill = rope_q.pop(0)
                        rot_ps = prot.tile([HD, SC], f32)
                        nc.tensor.matmul(rot_ps[:], rt_sb[:], raw[:],
                                         start=True, stop=True)
                        tmp = pr.tile([HD, SC], f32, tag="rope_tmp")
                        nc.vector.tensor_mul(tmp[:], raw[:].bitcast(f32),
                                             cs_sb[:])
                        e1 = pr.tile([HD, SC], f32, tag="rope_e1")
                        nc.vector.tensor_mul(e1[:], rot_ps[:], sn_sb[:])
                        nc.vector.tensor_add(out_ap, tmp[:], e1[:])
                        if spill is not None:
                            nc.sync.dma_start(spill, out_ap)

                    # K projection + rope -> resident SBUF
                    for ft in range(KVPC):
                        fsl = slice(ft * HD, (ft + 1) * HD)
                        k_ps = pps.tile([HD, SC], f32, tag="qk_ps")
                        for kt in range(NKT):
                            nc.tensor.matmul(k_ps[:], wk_sb[:, kt, fsl], xck(kt),
                                             start=(kt == 0), stop=(kt == NKT - 1))
                        if ft > 0:
                            rope_flush()
                        rope_start(k_ps, kt_sb[:, ft, ssl])

                    # V projection, token-major -> resident SBUF
                    for st in range(SC // HD):
                        tsl = slice(st * HD, (st + 1) * HD)
                        v_ps = pvps.tile([HD, FKV], f32, tag="v_ps")
                        for kt in range(NKT):
                            nc.tensor.matmul(v_ps[:], xck(kt)[:, tsl], wv_sb[:, kt, :],
                                             start=(kt == 0), stop=(kt == NKT - 1))
                        rope_flush()
                        nc.any.tensor_copy(
                            out=v_sb[:, sc * (SC // HD) + st, :], in_=v_ps[:])

                    # prefetch next chunk's first half into the free slot
                    if sc + 1 < NSC:
                        pending[(sc + 1, 0)] = load_half(sc + 1, 0)

                    # Q projection (fp8 DoubleRow: 256-deep contraction per
                    # instruction at ~2x PE rate) + rope -> spill to DRAM
                    for ft in range(HPC):
                        fsl = slice(ft * HD, (ft + 1) * HD)
                        q_ps = pps.tile([HD, SC], f32, tag="qk_ps")
                        for kp in range(NKT // 2):
                            nc.tensor.matmul(
                                q_ps[:], wq_sb[:, 2 * kp:2 * kp + 2, fsl],
                                x8c[:, 2 * kp:2 * kp + 2, :],
                                start=(kp == 0), stop=(kp == NKT // 2 - 1),
                                perf_mode=DR)
                        rope_flush()
                        qfin = pr.tile([HD, SC], bf16, tag="qfin", bufs=4)
                        rope_start(q_ps, qfin[:], spill=qt_ds[sc][fsl, :])
                    rope_flush()

                    if sc + 1 < NSC:
                        pending[(sc + 1, 1)] = load_half(sc + 1, 1)
                        pending[(sc + 1, "x8")] = load_x8(sc + 1)

        # Phase C weights: pool opened before phase B so the wo load (8.4 MB)
        # overlaps attention compute.
        with tc.tile_pool(name="pc_w", bufs=1) as pcw:
            wo_sb = pcw.tile([HD, HPC, DIM], f32r)

            # ------- Phase B+C fused: causal attention + out-projection -----
            # q-chunk outer, head-pair inner; each chunk's attention output
            # stays resident in SBUF and its output projection is emitted
            # right behind it, so the PE drains the scalar engine's exp
            # backlog with projection matmuls instead of idling, and the
            # ot spill/reload round-trip disappears.
            with tc.tile_pool(name="pb_c", bufs=1) as pbc, \
                 tc.tile_pool(name="pb_q", bufs=4) as pbq, \
                 tc.tile_pool(name="pb_e", bufs=8) as pbe, \
                 tc.tile_pool(name="pb_o", bufs=3) as pbo, \
                 tc.tile_pool(name="pb_sps", bufs=2, space="PSUM") as sps, \
                 tc.tile_pool(name="pb_ops", bufs=2, space="PSUM") as ops, \
                 tc.tile_pool(name="pb_bps", bufs=2, space="PSUM") as bps:

                # qt tiles are prefetched one (qc, h0) iteration ahead so the
                # first scores matmul of each block never waits on the DMA
                qt_pending = {}

                def load_qt(h, qc):
                    qt = pbq.tile([HD, SC], bf16, tag="qt",
                                  name=f"qt{h}_{qc}")
                    nc.sync.dma_start(
                        qt[:], qt_ds[qc][h * HD:(h + 1) * HD, :])
                    qt_pending[(h, qc)] = qt

                for h in (0, 1):
                    load_qt(h, 0)
                on_sb = pbc.tile([HD, HD], bf16)
                mk_sb = pbc.tile([HD, HD], bf16)
                nc.sync.dma_start(on_sb[:], on_d[:])
                nc.sync.dma_start(mk_sb[:], mk_d[:, 0, 0:HD])
                for qc in range(NSC):
                    otr = pbo.tile([HD, HPC, SC], f32r, tag="ot_res", bufs=2,
                                   name=f"otr{qc}")
                    for h0 in range(0, HPC, 2):
                        hs = (h0, h0 + 1)
                        kv = h0 // (HPC // KVPC)
                        if qc == 0:
                            # spread the wo load across the first chunk's
                            # blocks; all of it lands before C(0) needs it
                            for h in hs:
                                nc.sync.dma_start(
                                    wo_sb[:, h, :],
                                    wo_d[h * HD:(h + 1) * HD, :])
                        nkt = 4 * qc + 4
                        npairs = nkt // 2
                        qts, ots, bcs, es, epq = {}, {}, {}, {}, {}
                        for h in hs:
                            qts[h] = qt_pending.pop((h, qc))
                            ots[h] = ops.tile([HD, SC], f32, tag="ot_ps",
                                              name=f"ot{h}")
                            bcs[h] = bps.tile([HD, SC], f32, tag="bc_ps",
                                              name=f"bc{h}")
                        # prefetch the next iteration's q tiles
                        nqc, nh0 = (qc, h0 + 2) if h0 + 2 < HPC else (qc + 1, 0)
                        if nqc < NSC:
                            for h in (nh0, nh0 + 1):
                                load_qt(h, nqc)

                        def b_scores(h, kp):
                            """scores + exp (+mask) for k-tile pair kp."""
                            kt0, kt1 = 2 * kp, 2 * kp + 1
                            qt = qts[h]
                            s_ps = sps.tile([HD, 2, SC], f32, tag="s_ps")
                            e = pbe.tile([HD, 2, SC], bf16, tag="e")
                            es[(h, kp)] = e
                            if kt1 < 4 * qc:
                                # off-diagonal pair: full width, one exp
                                for i, kt in enumerate((kt0, kt1)):
                                    nc.tensor.matmul(
                                        s_ps[:, i, :],
                                        kt_sb[:, kv, kt * HD:(kt + 1) * HD],
                                        qt[:], start=True, stop=True)
                                nc.scalar.activation(e[:], s_ps[:], Exp,
                                                     scale=SCALE / WQS)
                            else:
                                q00 = (kt0 - 4 * qc) * HD
                                for i, kt in enumerate((kt0, kt1)):
                                    q0 = (kt - 4 * qc) * HD
                                    nc.tensor.matmul(
                                        s_ps[:, i, q0:],
                                        kt_sb[:, kv, kt * HD:(kt + 1) * HD],
                                        qt[:, q0:], start=True, stop=True)
                                # one exp covers both tiles from the first
                                # tile's causal start; the second tile's
                                # [q00:q00+HD) columns hold junk exp of stale
                                # psum, but every consumer (PV, quad-sum,
                                # mask) is ranged past them
                                nc.scalar.activation(
                                    e[:, :, q00:], s_ps[:, :, q00:], Exp,
                                    scale=SCALE / WQS)
                                for i, kt in enumerate((kt0, kt1)):
                                    q0 = (kt - 4 * qc) * HD
                                    # intra-tile triangle mask (in place)
                                    nc.vector.tensor_mul(
                                        e[:, i, q0:q0 + HD],
                                        e[:, i, q0:q0 + HD],
                                        mk_sb[:])

                        def b_accum(h, kp):
                            """P@V + denominator accumulation for pair kp.

                            Denominator: DVE sums four e-tiles (one quad = two
                            pairs) into one bf16 ep, so the ones-matmul runs
                            once per quad at full width; quads never straddle
                            the diagonal (it starts at kt = 4*qc, a multiple
                            of 4), so only the diagonal quad needs ranged adds.
                            """
                            kt0, kt1 = 2 * kp, 2 * kp + 1
                            e = es.pop((h, kp))
                            for i, kt in enumerate((kt0, kt1)):
                                q0 = max(0, (kt - 4 * qc) * HD)
                                nc.tensor.matmul(
                                    ots[h][:, q0:],
                                    v_sb[:, kt, kv * HD:(kv + 1) * HD],
                                    e[:, i, q0:], start=(kt == 0),
                                    stop=(kt == nkt - 1),
                                    skip_group_check=True)
                            g0 = max(0, (kt0 - 4 * qc) * HD)
                            g1 = max(0, (kt1 - 4 * qc) * HD)
                            if kp % 2 == 0:
                                ep = pbe.tile([HD, SC], bf16, tag="ep")
                                epq[h] = ep
                                if g1 == 0:
                                    nc.vector.tensor_add(
                                        ep[:], e[:, 0, :], e[:, 1, :])
                                else:
                                    nc.vector.tensor_copy(
                                        out=ep[:, g0:], in_=e[:, 0, g0:])
                                    nc.vector.tensor_add(
                                        ep[:, g1:], ep[:, g1:], e[:, 1, g1:])
                            else:
                                ep = epq.pop(h)
                                nc.vector.tensor_add(
                                    ep[:, g0:], ep[:, g0:], e[:, 0, g0:])
                                nc.vector.tensor_add(
                                    ep[:, g1:], ep[:, g1:], e[:, 1, g1:])
                                nc.tensor.matmul(
                                    bcs[h][:], on_sb[:], ep[:],
                                    start=kp == 1, stop=kp == npairs - 1,
                                    skip_group_check=True)

                        # two heads interleaved, scores one pair ahead: while
                        # the scalar engine exps one head's pair, the PE runs
                        # the other head's matmuls
                        b_scores(h0, 0)
                        b_scores(h0 + 1, 0)
                        for kp in range(npairs):
                            for h in hs:
                                if kp + 1 < npairs:
                                    b_scores(h, kp + 1)
                                b_accum(h, kp)

                        for h in hs:
                            inv = pbo.tile([HD, SC], f32, tag="inv")
                            nc.vector.reciprocal(inv[:], bcs[h][:])
                            nc.vector.tensor_mul(otr[:, h, :], ots[h][:],
                                                 inv[:])

                    # ---- output projection for this q-chunk (fused) ----
                    # y_ps shares the s_ps psum tag; y accumulates a full
                    # [128, DIM] row block in SBUF so the store is one DMA
                    # per q-tile
                    for qs in range(SC // HD):
                        qt0 = qc * SC + qs * HD
                        y_sb = pbo.tile([HD, DIM], bf16, tag="y_sb", bufs=2)
                        for dc in range(DIM // SC):
                            dsl = slice(dc * SC, (dc + 1) * SC)
                            y_ps = sps.tile([HD, 2, SC], f32, tag="s_ps",
                                            name=f"y{qs}_{dc}")
                            for ft in range(HPC):
                                nc.tensor.matmul(
                                    y_ps[:, 0, :],
                                    otr[:, ft, qs * HD:(qs + 1) * HD],
                                    wo_sb[:, ft, dsl],
                                    start=(ft == 0), stop=(ft == HPC - 1))
                            # keep y evacuation off the scalar engine: ACT is
                            # the pacing engine in the fused attention tail
                            nc.vector.tensor_copy(out=y_sb[:, dsl],
                                                  in_=y_ps[:, 0, :])
                        nc.sync.dma_start(y_d[qt0:qt0 + HD, :], y_sb[:])

    nc.finalize()
    return nc


def _rope_perm(nheads):
    """Row permutation putting even dims first within each head."""
    idx = []
    for h in range(nheads):
        base = h * HD
        idx.extend(base + 2 * j for j in range(HD // 2))
        idx.extend(base + 2 * j + 1 for j in range(HD // 2))
    return np.array(idx)


def _prepare_in_maps(inputs):
    import ml_dtypes
    ml_bf16 = ml_dtypes.bfloat16
    ml_f8 = ml_dtypes.float8_e4m3
    x = np.ascontiguousarray(np.asarray(inputs["x"], dtype=np.float32))
    fc = np.asarray(inputs["freqs_cos"], dtype=np.float32)
    fs = np.asarray(inputs["freqs_sin"], dtype=np.float32)
    wq = np.asarray(inputs["wq"], dtype=np.float32)
    wk = np.asarray(inputs["wk"], dtype=np.float32)
    wv = np.asarray(inputs["wv"], dtype=np.float32)
    wo = np.asarray(inputs["wo"], dtype=np.float32)
    aq = np.asarray(inputs["aq"], dtype=np.float32)
    bq = np.asarray(inputs["bq"], dtype=np.float32)
    ak = np.asarray(inputs["ak"], dtype=np.float32)
    bk = np.asarray(inputs["bk"], dtype=np.float32)
    av = np.asarray(inputs["av"], dtype=np.float32)
    bv = np.asarray(inputs["bv"], dtype=np.float32)
    ao = np.asarray(inputs["ao"], dtype=np.float32)
    bo = np.asarray(inputs["bo"], dtype=np.float32)

    permQ = _rope_perm(HPC)
    permK = _rope_perm(KVPC)
    # fold LoRA into dense weights: W_eff = W + B @ A
    wq = wq + bq.astype(np.float64) @ aq.astype(np.float64)
    wk = wk + bk.astype(np.float64) @ ak.astype(np.float64)
    wv = wv + bv.astype(np.float64) @ av.astype(np.float64)
    wo = wo + bo.astype(np.float64) @ ao.astype(np.float64)
    wq = wq.astype(np.float32)
    wk = wk.astype(np.float32)
    wv = wv.astype(np.float32)
    wo = wo.astype(np.float32)
    fcT = np.ascontiguousarray(fc.T)                       # [64, S]
    fsT = np.ascontiguousarray(fs.T)
    cs = np.concatenate([fcT, fcT], axis=0)                # [128, S]
    sn = np.concatenate([fsT, fsT], axis=0)
    rt = np.zeros((HD, HD), np.float32)
    for j in range(HD // 2):
        rt[j, 64 + j] = 1.0      # (R^T)[j, 64+j] = R[64+j, j] = +1
        rt[64 + j, j] = -1.0     # (R^T)[64+j, j] = R[j, 64+j] = -1
    ones = np.ones((HD, HD), np.float32)
    kk = np.arange(HD)[:, None]
    qq = np.arange(SC)[None, :]
    mk = np.stack([(qq >= (128 * r + kk)).astype(np.float32) for r in range(4)],
                  axis=1)                                  # [128, 4, SC]

    xt_cache = {}
    xt8_cache = {}
    in_maps = []
    for c in range(8):
        b, g = c // 2, c % 2
        if b not in xt_cache:
            xt_cache[b] = np.ascontiguousarray(x[b].T)
            xt8_cache[b] = np.clip(xt_cache[b], -240.0, 240.0).astype(ml_f8)
        fq = slice(g * FQ, (g + 1) * FQ)
        fkv = slice(g * FKV, (g + 1) * FKV)
        wq_g = wq[fq][permQ]
        wk_g = wk[fkv][permK]
        in_maps.append({
            "xt": xt_cache[b],
            "xt8": xt8_cache[b],
            "wq8": np.clip(np.ascontiguousarray(wq_g.T) * WQS,
                           -240.0, 240.0).astype(ml_f8),
            "wk": np.ascontiguousarray(wk_g.T),
            "wv": np.ascontiguousarray(wv[fkv].T),
            "wo": np.ascontiguousarray(wo[:, fq].T),
            "cs": cs.astype(ml_bf16), "sn": sn.astype(ml_bf16),
            "rt": rt, "on": ones.astype(ml_bf16), "mk": mk.astype(ml_bf16),
        })
    return in_maps


def _get_program():
    if "nc" not in _cache:
        _cache["nc"] = _build_program()
    return _cache["nc"]


def run(inputs, trace=False):
    from concourse import bass_utils
    nc = _get_program()
    in_maps = _prepare_in_maps(inputs)
    res = bass_utils.run_bass_kernel_spmd(
        nc, in_maps, list(range(8)), trace=trace)
    ys = [res.results[c]["y"] for c in range(8)]
    out = np.empty((B, S, DIM), np.float32)
    for b in range(B):
        out[b] = ys[2 * b] + ys[2 * b + 1]
    return out, res


def kernel(**inputs):
    out, _ = run(inputs, trace=False)
    return out


def bench_chain(inputs, nlo=4, nhi=36, rounds=7, n_cores=8):
    """Marginal per-execution device time via chained NEFF executions.

    Builds one jitted program that runs the kernel N times back-to-back on
    device (each execution's output buffers feed the next, forcing serial
    order), so the host/axon dispatch overhead is paid once per program.
    Marginal time (wall(nhi) - wall(nlo)) / (nhi - nlo) is the true
    per-execution time.  Returns (median_marginal_seconds, output).
    """
    import time

    import jax
    import concourse.mybir as mybir
    from concourse import bass2jax
    from concourse.bass2jax import _bass_exec_p, partition_id_tensor
    from jax.sharding import Mesh, NamedSharding, PartitionSpec
    from jax.experimental.shard_map import shard_map

    bass2jax.install_neuronx_cc_hook()
    nc = _get_program()
    in_maps = _prepare_in_maps(inputs)

    partition_name = nc.partition_id_tensor.name if nc.partition_id_tensor else None
    in_names, out_names, out_avals = [], [], []
    for alloc in nc.m.functions[0].allocations:
        if not isinstance(alloc, mybir.MemoryLocationSet):
            continue
        name = alloc.memorylocations[0].name
        if alloc.kind == "ExternalInput":
            if name != partition_name:
                in_names.append(name)
        elif alloc.kind == "ExternalOutput":
            out_names.append(name)
            out_avals.append(jax.core.ShapedArray(
                tuple(alloc.tensor_shape), mybir.dt.np(alloc.dtype)))
    n_params = len(in_names)
    all_names = list(in_names) + out_names
    if partition_name is not None:
        all_names.append(partition_name)

    devices = jax.devices()[:n_cores]
    mesh = Mesh(np.asarray(devices), ("core",))
    spec = NamedSharding(mesh, PartitionSpec("core"))

    def make(n):
        def _body(*args):
            ins = list(args[:n_params])
            outs = list(args[n_params:])
            for _ in range(n):
                operands = ins + outs
                if partition_name is not None:
                    operands.append(partition_id_tensor())
                outs = list(_bass_exec_p.bind(
                    *operands,
                    out_avals=tuple(out_avals),
                    in_names=tuple(all_names),
                    out_names=tuple(out_names),
                    lowering_input_output_aliases=(),
                    sim_require_finite=True,
                    sim_require_nnan=True,
                    nc=nc,
                ))
            return tuple(outs)

        return jax.jit(shard_map(
            _body, mesh=mesh,
            in_specs=(PartitionSpec("core"),) * (n_params + len(out_names)),
            out_specs=(PartitionSpec("core"),) * len(out_names),
            check_rep=False), keep_unused=True)

    concat_in = [
        jax.device_put(
            np.concatenate([np.asarray(in_maps[c][nm]) for c in range(n_cores)],
                           axis=0), spec)
        for nm in in_names]
    concat_zeros = [
        jax.device_put(
            np.zeros((n_cores * a.shape[0], *a.shape[1:]), a.dtype), spec)
        for a in out_avals]

    flo, fhi = make(nlo), make(nhi)
    out = flo(*concat_in, *concat_zeros)
    jax.block_until_ready(out)
    jax.block_until_ready(fhi(*concat_in, *concat_zeros))

    margs = []
    for _ in range(rounds):
        t0 = time.perf_counter()
        jax.block_until_ready(fhi(*concat_in, *concat_zeros))
        t1 = time.perf_counter()
        jax.block_until_ready(flo(*concat_in, *concat_zeros))
        t2 = time.perf_counter()
        margs.append(((t1 - t0) - (t2 - t1)) / (nhi - nlo))
    med = float(np.median(margs))

    ys = np.asarray(out[out_names.index("y")]).reshape(n_cores, S, DIM)
    full = np.empty((B, S, DIM), np.float32)
    for b in range(B):
        full[b] = ys[2 * b] + ys[2 * b + 1]
    return med, full, [round(m * 1e9) for m in margs]


def bench(inputs, iters=20, n_cores=8):
    """Time repeated NEFF executions with device-resident inputs.

    Mirrors bass2jax.run_bass_via_pjrt's multi-core path without donation so
    buffers can be reused across calls.  Returns (avg_exec_seconds, output).
    """
    import time

    import jax
    import concourse.mybir as mybir
    from concourse import bass2jax
    from concourse.bass2jax import _bass_exec_p, partition_id_tensor
    from jax.sharding import Mesh, NamedSharding, PartitionSpec

    bass2jax.install_neuronx_cc_hook()
    nc = _get_program()
    in_maps = _prepare_in_maps(inputs)

    partition_name = nc.partition_id_tensor.name if nc.partition_id_tensor else None
    in_names, out_names, out_avals = [], [], []
    for alloc in nc.m.functions[0].allocations:
        if not isinstance(alloc, mybir.MemoryLocationSet):
            continue
        name = alloc.memorylocations[0].name
        if alloc.kind == "ExternalInput":
            if name != partition_name:
                in_names.append(name)
        elif alloc.kind == "ExternalOutput":
            out_names.append(name)
            out_avals.append(jax.core.ShapedArray(
                tuple(alloc.tensor_shape), mybir.dt.np(alloc.dtype)))
    n_params = len(in_names)
    all_names = list(in_names) + out_names
    if partition_name is not None:
        all_names.append(partition_name)

    def _body(*args):
        operands = list(args)
        if partition_name is not None:
            operands.append(partition_id_tensor())
        outs = _bass_exec_p.bind(
            *operands,
            out_avals=tuple(out_avals),
            in_names=tuple(all_names),
            out_names=tuple(out_names),
            lowering_input_output_aliases=(),
            sim_require_finite=True,
            sim_require_nnan=True,
            nc=nc,
        )
        return tuple(outs)

    devices = jax.devices()[:n_cores]
    mesh = Mesh(np.asarray(devices), ("core",))
    spec = NamedSharding(mesh, PartitionSpec("core"))
    from jax.experimental.shard_map import shard_map
    sharded = jax.jit(shard_map(
        _body, mesh=mesh,
        in_specs=(PartitionSpec("core"),) * (n_params + len(out_names)),
        out_specs=(PartitionSpec("core"),) * len(out_names),
        check_rep=False), keep_unused=True)

    concat_in = [
        jax.device_put(
            np.concatenate([np.asarray(in_maps[c][nm]) for c in range(n_cores)],
                           axis=0), spec)
        for nm in in_names]
    concat_zeros = [
        jax.device_put(
            np.zeros((n_cores * a.shape[0], *a.shape[1:]), a.dtype), spec)
        for a in out_avals]
    out = sharded(*concat_in, *concat_zeros)
    jax.block_until_ready(out)
    t0 = time.perf_counter()
    for _ in range(iters):
        out = sharded(*concat_in, *concat_zeros)
    jax.block_until_ready(out)
    t1 = time.perf_counter()

    if n_cores != 8:
        return (t1 - t0) / iters, None
    ys = np.asarray(out[out_names.index("y")]).reshape(n_cores, S, DIM)
    full = np.empty((B, S, DIM), np.float32)
    for b in range(B):
        full[b] = ys[2 * b] + ys[2 * b + 1]
    return (t1 - t0) / iters, full



